# revision 1
# baseline (speedup 1.0000x reference)
"""Trainium2 Bass kernel for a 4-layer GCN (N=50000, D=128, E=1600000, 8 cores).

Strategy (graph/data parallel over destination nodes):
  - Nodes padded to 50176 = 392*128; each of 8 cores owns 6272 nodes (49 tiles).
  - Per layer, linearity lets us reorder:  out = dis * ((sum_{e->v} x'[src_e]) @ W) + b + prev
    with x' = dis * x  (self-loops appended as ordinary edges).
  - The scatter-sum runs as: dma_gather of x' rows (bf16, edge-major) +
    one-hot fp8 scatter matrices S streamed from DRAM, accumulated on the PE:
       psum_aggT[fi,dst] += msg_chunk[e,fi]^T @ S_chunk[e,dst]
  - aggT (SBUF, bf16) is then lhsT of a second matmul with W moving ->
    node-major psum_out[dst,fo]; epilogue (dis scale, +b+prev, relu) on DVE/ACT.
  - x'_next slices are AllGather'd into a fresh full table for the next layer.
  - int16 gather indices => table split in two 25088-row halves (A/B calls).
All preprocessing (degree, norm, edge partitioning, one-hot S, padding) is done
host-side in numpy; the edge structure is baked into the instruction stream.
"""

import sys

sys.path.insert(0, "/opt/trn_rl_repo")

import numpy as np
import ml_dtypes

N = 50000
D = 128
L = 4
E = 1600000
NCORES = 8
NPAD = 50176  # 392 * 128
NPC = NPAD // NCORES  # 6272 nodes per core
TPC = NPC // 128  # 49 tiles per core
HALF = NPAD // 2  # 25088, int16-indexable halves
GROUP = 4  # dst tiles per gather call pair
NGROUPS = (TPC + GROUP - 1) // GROUP  # 13

_compiled = None


def _preprocess(x, edge_index, W, b):
    src0 = edge_index[0].astype(np.int64)
    dst0 = edge_index[1].astype(np.int64)
    loops = np.arange(N, dtype=np.int64)
    src = np.concatenate([src0, loops])
    dst = np.concatenate([dst0, loops])
    deg = np.bincount(dst, minlength=N).astype(np.float32)
    dis = np.zeros(NPAD, np.float32)
    dis[:N] = 1.0 / np.sqrt(deg)

    # order edges by (core, local tile, half) once
    core_of = dst // NPC
    lt_of = (dst % NPC) // 128
    dloc_of = dst % 128
    half_of = (src >= HALF).astype(np.int64)
    key = ((core_of * TPC + lt_of) * 2 + half_of)
    order = np.argsort(key, kind="stable")
    src_s, key_s = src[order], key[order]
    dloc_s = dloc_of[order]
    # counts per (core, lt, half)
    counts = np.bincount(key_s, minlength=NCORES * TPC * 2).reshape(NCORES, TPC, 2)
    # shared chunk structure: per (lt, half) max over cores
    KCH = np.ceil(counts.max(axis=0) / 128.0).astype(np.int64)  # [TPC, 2]
    KCH = np.maximum(KCH, 1)

    # group layout (shared across cores): per group g covers tiles [g*GROUP, ...)
    # chunk stream order: g0(A tiles..., B tiles...), g1(...)
    tiles_of_group = [list(range(g * GROUP, min((g + 1) * GROUP, TPC))) for g in range(NGROUPS)]
    chA = [int(sum(KCH[t, 0] for t in tg)) for tg in tiles_of_group]
    chB = [int(sum(KCH[t, 1] for t in tg)) for tg in tiles_of_group]
    TOTCH = int(sum(chA) + sum(chB))

    # per-(lt,half) chunk offset in the global stream
    chunk_off = np.zeros((TPC, 2), np.int64)
    pos = 0
    for g, tg in enumerate(tiles_of_group):
        for h in (0, 1):
            for t in tg:
                chunk_off[t, h] = pos
                pos += KCH[t, h]
    assert pos == TOTCH

    # build per-core idx + S
    idx_all = np.zeros((NCORES, TOTCH * 128), np.int16)
    dlocs = np.full((NCORES, TOTCH * 128), -1, np.int16)
    starts = np.zeros(NCORES * TPC * 2 + 1, np.int64)
    np.cumsum(counts.reshape(-1), out=starts[1:])
    for c in range(NCORES):
        for t in range(TPC):
            for h in (0, 1):
                k = (c * TPC + t) * 2 + h
                s0, s1 = starts[k], starts[k + 1]
                n = s1 - s0
                o = chunk_off[t, h] * 128
                iv = src_s[s0:s1] - (HALF if h else 0)
                idx_all[c, o:o + n] = iv.astype(np.int16)
                dlocs[c, o:o + n] = dloc_s[s0:s1]

    # S matrices: [TOTCH, 128, 128] fp8, row e -> column dloc (skip -1)
    smat = np.zeros((NCORES, TOTCH, 128, 128), ml_dtypes.float8_e4m3)
    ii = np.arange(TOTCH * 128)
    for c in range(NCORES):
        d = dlocs[c]
        m = d >= 0
        smat[c][ii[m] // 128, ii[m] % 128, d[m]] = 1.0

    # idx tiles wrapped: [128, TOTCH*128//16], element i at [i%16, i//16], x8 groups
    idxw = np.zeros((NCORES, 128, TOTCH * 8), np.int16)
    for c in range(NCORES):
        w = idx_all[c].reshape(TOTCH * 8, 16).T
        for gme in range(8):
            idxw[c, gme * 16:(gme + 1) * 16, :] = w

    xpad = np.zeros((NPAD, D), np.float32)
    xpad[:N] = x
    x0p = (xpad * dis[:, None]).astype(ml_dtypes.bfloat16)
    disT = dis.reshape(NCORES, TPC, 128).transpose(0, 2, 1).copy()  # [c,128,TPC]
    b_bc = np.broadcast_to(b[None, :, :], (128, L, D)).astype(np.float32).copy()

    meta = dict(KCH=KCH, chA=chA, chB=chB, TOTCH=TOTCH, chunk_off=chunk_off,
                tiles_of_group=tiles_of_group)
    per_core = []
    for c in range(NCORES):
        per_core.append(dict(
            x_own=np.ascontiguousarray(xpad[c * NPC:(c + 1) * NPC]),
            x0p=x0p,
            w=W.astype(ml_dtypes.bfloat16),
            b_bc=b_bc,
            disT=np.ascontiguousarray(disT[c]),
            idxs=np.ascontiguousarray(idxw[c]),
            smat=np.ascontiguousarray(smat[c].reshape(TOTCH, 128, 128)),
        ))
    return meta, per_core


def _build(meta):
    from concourse import bacc, tile, bass_utils
    from concourse.bass import mybir

    KCH = meta["KCH"]
    chA, chB = meta["chA"], meta["chB"]
    TOTCH = meta["TOTCH"]
    chunk_off = meta["chunk_off"]
    tiles_of_group = meta["tiles_of_group"]
    MAXCH_G = max(chA[g] + chB[g] for g in range(NGROUPS))

    nc = bacc.Bacc("TRN2", target_bir_lowering=False, debug=False,
                   num_devices=NCORES)
    d_x_own = nc.dram_tensor("x_own", [NPC, D], mybir.dt.float32, kind="ExternalInput")
    d_x0p = nc.dram_tensor("x0p", [NPAD, D], mybir.dt.bfloat16, kind="ExternalInput")
    d_w = nc.dram_tensor("w", [L, D, D], mybir.dt.bfloat16, kind="ExternalInput")
    d_bbc = nc.dram_tensor("b_bc", [128, L, D], mybir.dt.float32, kind="ExternalInput")
    d_disT = nc.dram_tensor("disT", [128, TPC], mybir.dt.float32, kind="ExternalInput")
    d_idxs = nc.dram_tensor("idxs", [128, TOTCH * 8], mybir.dt.int16, kind="ExternalInput")
    d_smat = nc.dram_tensor("smat", [TOTCH, 128, 128], mybir.dt.float8e4, kind="ExternalInput")
    d_out = nc.dram_tensor("x_out", [NPC, D], mybir.dt.float32, kind="ExternalOutput")

    with tile.TileContext(nc) as tc:
        with (
            tc.tile_pool(name="const", bufs=1) as constp,
            tc.tile_pool(name="msg", bufs=2) as msgp,
            tc.tile_pool(name="sp", bufs=2) as sp_pool,
            tc.tile_pool(name="work", bufs=3) as workp,
            tc.tile_pool(name="xprime", bufs=3) as xpp,
            tc.tile_pool(name="pag", bufs=2, space="PSUM") as pagp,
            tc.tile_pool(name="pout", bufs=2, space="PSUM") as poutp,
            tc.tile_pool(name="dram", bufs=1, space="DRAM") as dramp,
        ):
            # ---- persistent SBUF state ----
            x_own = constp.tile([128, TPC, D], mybir.dt.float32, tag="x_own")
            nc.sync.dma_start(out=x_own[:], in_=d_x_own.ap().rearrange("(t p) f -> p t f", p=128))
            w_sb = constp.tile([128, L, D], mybir.dt.bfloat16, tag="w_sb")
            nc.sync.dma_start(out=w_sb[:], in_=d_w.ap().rearrange("l k f -> k l f"))
            bbc_sb = constp.tile([128, L, D], mybir.dt.float32, tag="bbc")
            nc.sync.dma_start(out=bbc_sb[:], in_=d_bbc.ap())
            disT_sb = constp.tile([128, TPC], mybir.dt.float32, tag="disT")
            nc.sync.dma_start(out=disT_sb[:], in_=d_disT.ap())
            idx_sb = constp.tile([128, TOTCH * 8], mybir.dt.int16, tag="idx")
            nc.sync.dma_start(out=idx_sb[:], in_=d_idxs.ap())

            # DRAM tables for layers 1..3 and AllGather input slices
            tables = [d_x0p.ap()]
            ag_ins = []
            for l in range(1, L):
                tab_tile = dramp.tile([NPAD, D], mybir.dt.bfloat16, tag=f"tab{l}")
                agin_tile = dramp.tile([NPC, D], mybir.dt.bfloat16, tag=f"agin{l}")
                tables.append(tab_tile[:])
                ag_ins.append(agin_tile[:])

            for l in range(L):
                table = tables[l]
                tabA = table[0:HALF, :]
                tabB = table[HALF:NPAD, :]
                for g in range(NGROUPS):
                    tg = tiles_of_group[g]
                    nA, nB = chA[g], chB[g]
                    goff = chunk_off[tg[0], 0]  # stream offset of this group
                    # S for the whole group in one DMA
                    s_t = sp_pool.tile([128, MAXCH_G, 128], mybir.dt.float8e4, tag="s_t")
                    nc.sync.dma_start(
                        out=s_t[:, 0:nA + nB, :],
                        in_=d_smat.ap()[goff:goff + nA + nB].rearrange("c p f -> p c f"),
                    )
                    # gather msg rows for both halves
                    msg = msgp.tile([128, MAXCH_G, D], mybir.dt.bfloat16, tag="msg")
                    if nA > 0:
                        nc.gpsimd.dma_gather(
                            out_ap=msg[:, 0:nA, :],
                            in_ap=tabA,
                            idxs_ap=idx_sb[:, goff * 8:(goff + nA) * 8],
                            num_idxs=nA * 128,
                            num_idxs_reg=nA * 128,
                            elem_size=D,
                            single_packet=False,
                        )
                    if nB > 0:
                        nc.gpsimd.dma_gather(
                            out_ap=msg[:, nA:nA + nB, :],
                            in_ap=tabB,
                            idxs_ap=idx_sb[:, (goff + nA) * 8:(goff + nA + nB) * 8],
                            num_idxs=nB * 128,
                            num_idxs_reg=nB * 128,
                            elem_size=D,
                            single_packet=False,
                        )
                    # per dst tile: accumulate one-hot matmuls, then @W + epilogue
                    for t in tg:
                        kA = int(KCH[t, 0])
                        kB = int(KCH[t, 1])
                        oA = int(chunk_off[t, 0] - goff)
                        oB = int(chunk_off[t, 1] - goff)
                        psA = pagp.tile([128, 128], mybir.dt.float32, tag="pag")
                        nchunks = kA + kB
                        ci = 0
                        for c in range(kA):
                            nc.tensor.matmul(psA[:], lhsT=msg[:, oA + c, :],
                                             rhs=s_t[:, oA + c, :],
                                             start=(ci == 0), stop=(ci == nchunks - 1))
                            ci += 1
                        for c in range(kB):
                            nc.tensor.matmul(psA[:], lhsT=msg[:, oB + c, :],
                                             rhs=s_t[:, oB + c, :],
                                             start=(ci == 0), stop=(ci == nchunks - 1))
                            ci += 1
                        aggT = workp.tile([128, 128], mybir.dt.bfloat16, tag="aggT")
                        nc.scalar.copy(aggT[:], psA[:])
                        pso = poutp.tile([128, 128], mybir.dt.float32, tag="pout")
                        nc.tensor.matmul(pso[:], lhsT=aggT[:], rhs=w_sb[:, l, :],
                                         start=True, stop=True)
                        # epilogue: xn = relu(dis*pso + b + prev); x' = bf16(dis*xn)
                        prevb = workp.tile([128, 128], mybir.dt.float32, tag="prevb")
                        nc.vector.tensor_tensor(
                            out=prevb[:], in0=x_own[:, t, :], in1=bbc_sb[:, l, :],
                            op=mybir.AluOpType.add)
                        t2 = workp.tile([128, 128], mybir.dt.float32, tag="t2")
                        nc.vector.tensor_scalar(
                            out=t2[:], in0=pso[:], scalar1=disT_sb[:, t:t + 1],
                            scalar2=None, op0=mybir.AluOpType.mult)
                        t3 = workp.tile([128, 128], mybir.dt.float32, tag="t3")
                        nc.vector.tensor_tensor(
                            out=t3[:], in0=t2[:], in1=prevb[:], op=mybir.AluOpType.add)
                        nc.scalar.activation(
                            out=x_own[:, t, :], in_=t3[:],
                            func=mybir.ActivationFunctionType.Relu)
                        if l < L - 1:
                            xpr = xpp.tile([128, 128], mybir.dt.bfloat16, tag="xpr")
                            nc.scalar.activation(
                                out=xpr[:], in_=x_own[:, t, :],
                                func=mybir.ActivationFunctionType.Copy,
                                scale=disT_sb[:, t:t + 1])
                            nc.sync.dma_start(
                                out=ag_ins[l].rearrange("(t p) f -> p t f", p=128)[:, t, :],
                                in_=xpr[:])
                if l < L - 1:
                    nc.gpsimd.collective_compute(
                        "AllGather",
                        mybir.AluOpType.bypass,
                        replica_groups=[list(range(NCORES))],
                        ins=[ag_ins[l].opt()],
                        outs=[tables[l + 1].opt()],
                    )
            nc.sync.dma_start(out=d_out.ap().rearrange("(t p) f -> p t f", p=128),
                              in_=x_own[:])

    nc.compile()
    return nc


def kernel(x, edge_index, W, b):
    global _compiled
    from concourse import bass_utils

    x = np.asarray(x, dtype=np.float32)
    W_np = np.asarray(W, dtype=np.float32)
    b_np = np.asarray(b, dtype=np.float32)
    ei = np.asarray(edge_index)

    meta, per_core = _preprocess(x, ei, W_np, b_np)
    globals()["_last_per_core"] = per_core
    if _compiled is None:
        _compiled = _build(meta)
    nc = _compiled
    res = bass_utils.run_bass_kernel_spmd(nc, per_core, core_ids=list(range(NCORES)))
    out = np.concatenate([res.results[c]["x_out"] for c in range(NCORES)], axis=0)
    return out[:N].astype(np.float32)

